# revision 40
# baseline (speedup 1.0000x reference)
# Trainium2 Bass kernel for the 2-layer GNN message-passing block.
# Self-contained: hardcodes shapes; takes full inputs, shards across 8 cores,
# returns the full [50000, 128] float32 output.
#
# Design (v3):
#  - Layer 0: no gather. Host streams x[src_e] feature-major (xeT); the device
#    computes GELU(W1^T x_e + b1) per edge, then accumulates the Ws1/We1/xd
#    terms into an EDGE-major PSUM (partition=edge) so the message GELU writes
#    scatter-ready SBUF directly (no PE transpose, single scalar op).
#  - Biases (mp b) are folded into an extra all-ones row on the edge-feature
#    stream: eT is [65, L], We' = [We; b].
#  - Layer 1: dma_gather (node-major rows) of xs1 from per-epoch AllGather
#    buffers at GOP=GRP=512 (one gather per compute group), consumed by a
#    single identity matmul into the edge-major PSUM. 6 front-loaded epochs
#    start the gather DMA early; gather emission is paced to keep GpSimd
#    responsive for AllGather triggers.
#  - Scatter per chunk via one-hot matmul into PSUM; Pt (the one-hot used for
#    the xd gather) is the PE-transpose of P4.
import os
import sys

sys.path.insert(0, "/opt/trn_rl_repo")

import numpy as np
import ml_dtypes

import concourse.bacc as bacc
import concourse.tile as tile
from concourse import mybir
from concourse.bass_utils import run_bass_kernel_spmd

BF16 = ml_dtypes.bfloat16
FP8 = ml_dtypes.float8_e4m3

N = 50000
NPAD = 50176
NC = 8
C = NPAD // NC            # 6272 nodes per core
WCNT = C // 128           # 49 windows of 128 nodes
NEP = 6                   # epochs (groups of local windows) for layer-1
EPW = [6, 8, 10, 11, 12, 2]   # windows per epoch (small first -> early AG)
EPW0 = [0, 6, 14, 24, 35, 47]
GRP = 512                 # edges per compute group (4 chunks) == gather op

F32 = mybir.dt.float32
BF = mybir.dt.bfloat16
F8 = mybir.dt.float8e4
I16 = mybir.dt.int16


def _bf(x):
    return np.ascontiguousarray(x.astype(BF16))


def _f8(x):
    return np.ascontiguousarray(np.asarray(x, np.float32).astype(FP8))


def _sort_stream(dst_local, sub, nsub, owner, extra_pad_unit):
    """Shared-layout edge stream sort.

    Edges keyed by (owner, sub, win). Returns per-core segment layout shared
    across cores (max counts), satisfying: each (sub, win) segment is a
    multiple of 128 edges (>=128), and each sub block is a multiple of
    extra_pad_unit edges.
    Returns (order, dest, seg info, L, chunk metadata arrays).
    """
    E = dst_local.shape[0]
    win = dst_local // 128
    key = (owner * nsub + sub) * WCNT + win
    order = np.argsort(key, kind="stable")
    ksort = key[order]
    counts_flat = np.bincount(key, minlength=NC * nsub * WCNT)
    counts = counts_flat.reshape(NC, nsub, WCNT)

    nch = np.maximum(1, -(-counts.max(axis=0) // 128)).astype(np.int64)  # [nsub, WCNT]
    seg_len = nch * 128
    sub_len = seg_len.sum(axis=1)                      # [nsub]
    sub_pad = (-sub_len) % extra_pad_unit
    sub_start = np.zeros(nsub, np.int64)
    pos = 0
    for s in range(nsub):
        sub_start[s] = pos
        pos += sub_len[s] + sub_pad[s]
    L = int(pos)

    seg_start = np.zeros((nsub, WCNT), np.int64)
    for s in range(nsub):
        p = sub_start[s]
        for w in range(WCNT):
            seg_start[s, w] = p
            p += seg_len[s, w]

    group_first = np.cumsum(counts_flat) - counts_flat
    within = np.arange(E, dtype=np.int64) - group_first[ksort]
    dest = seg_start[sub[order], win[order]] + within

    nchunks = L // 128
    cw = np.full(nchunks, -1, np.int64)     # window (-1 = pad chunk)
    cf = np.zeros(nchunks, bool)            # first chunk of segment
    cl = np.zeros(nchunks, bool)            # last chunk of segment
    cs = np.zeros(nchunks, np.int64)        # sub index
    for s in range(nsub):
        for w in range(WCNT):
            a = int(seg_start[s, w]) // 128
            n = int(nch[s, w])
            cw[a:a + n] = w
            cf[a] = True
            cl[a + n - 1] = True
            cs[a:a + n] = s
        pe = (int(sub_start[s]) + int(sub_len[s])) // 128
        pe2 = pe + int(sub_pad[s]) // 128
        cs[pe:pe2] = s
    return order, dest, L, cw, cf, cl, cs, sub_start, sub_len, sub_pad


def _prep(inputs):
    """Host-side graph partitioning / stream layout. Index+layout prep only."""
    src = np.asarray(inputs["edge_index"][0]).astype(np.int64)
    dst = np.asarray(inputs["edge_index"][1]).astype(np.int64)
    ef = np.asarray(inputs["edge_features"]).astype(np.float32)

    d_owner = dst // C
    dl = dst - d_owner * C

    s_owner = src // C
    s_lw = (src - s_owner * C) // 128
    s_ep = np.searchsorted(np.array(EPW0), s_lw, side="right") - 1

    # ---- layer-0 stream: sorted by (dst window) only ----
    z = np.zeros_like(dst)
    (o0, de0, L0, cw0, cf0, cl0, _, _, _, _) = _sort_stream(
        dl, z, 1, d_owner, 4 * GRP)

    # ---- layer-1 stream: sorted by (src epoch, dst window) ----
    (o1, de1, L1, cw1, cf1, cl1, cs1, sub_start1, sub_len1, sub_pad1) = \
        _sort_stream(dl, s_ep, NEP, d_owner, GRP)

    # epoch-buffer row index for every edge (gather idx within its epoch buf)
    ep_rows = np.array([EPW[g] * 128 for g in range(NEP)])
    g = s_ep
    ep0 = np.array(EPW0)[g]
    row = s_owner * ep_rows[g] + (s_lw - ep0) * 128 + (src - s_owner * C - s_lw * 128)
    assert row.max() < 32768

    x = np.asarray(inputs["x"]).astype(np.float32)
    xpad = np.zeros((NPAD, 128), np.float32)
    xpad[:N] = x
    x_bf = xpad.astype(BF16)

    iota = np.tile(np.arange(128, dtype=np.float32)[None, :], (128, 1))
    ident = np.eye(128, dtype=np.float32)

    def _we_bias(w, b):
        # [66, 128]: We with the bias folded in as an extra contraction row
        # (plus a zero row to keep the contraction partition count even)
        return _f8(np.vstack([np.asarray(w, np.float32),
                              np.asarray(b, np.float32)[None, :],
                              np.zeros((1, 128), np.float32)]))

    shared = {
        "W1": _bf(np.asarray(inputs["ff1_W"], np.float32)),
        "W1f8": _f8(np.asarray(inputs["ff1_W"], np.float32)),
        "Ws1": _bf(np.asarray(inputs["mp1_Wsrc"], np.float32)),
        "Wd1": _bf(np.asarray(inputs["mp1_Wdst"], np.float32)),
        "We1": _we_bias(inputs["mp1_We"], inputs["mp1_b"]),
        "Ws2": _bf(np.asarray(inputs["mp2_Wsrc"], np.float32)),
        "Wd2": _bf(np.asarray(inputs["mp2_Wdst"], np.float32)),
        "We2": _we_bias(inputs["mp2_We"], inputs["mp2_b"]),
        "W3": _bf(np.asarray(inputs["ff2_W"], np.float32)),
        "b1c": np.ascontiguousarray(
            np.asarray(inputs["ff1_b"], np.float32)[:, None]),
        "b3m": np.ascontiguousarray(
            np.tile(np.asarray(inputs["ff2_b"], np.float32)[None, :], (128, 1))),
        "iota": _bf(iota),
        "identb": _bf(ident),
    }

    per_core = []
    for c in range(NC):
        # layer-0 per-core stream (edge features + ones row for the bias)
        m0 = d_owner[o0] == c
        e0_ids = o0[m0]
        dp0 = de0[m0]
        eT0 = np.zeros((66, L0), np.float32)
        eT0[:64, dp0] = ef[e0_ids].T
        eT0[64, dp0] = 1.0
        xeT = np.zeros((128, L0), FP8)
        xeT[:, dp0] = xpad[src[e0_ids]].astype(FP8).T
        edc0 = np.full(L0, -1.0, np.float32)
        edc0[dp0] = dl[e0_ids] % 128

        # layer-1 per-core stream
        m1 = d_owner[o1] == c
        e1_ids = o1[m1]
        dp1 = de1[m1]
        eT1 = np.zeros((66, L1), np.float32)
        eT1[:64, dp1] = ef[e1_ids].T
        eT1[64, dp1] = 1.0
        edc1 = np.full(L1, -1.0, np.float32)
        edc1[dp1] = dl[e1_ids] % 128
        sidx = np.zeros(L1, np.int16)
        sidx[dp1] = row[e1_ids].astype(np.int16)

        per_core.append({
            "eT0": _f8(eT0),
            "xeT": np.ascontiguousarray(xeT),
            "edc0": _bf(np.ascontiguousarray(edc0.reshape(L0 // 128, 128).T)),
            "eT1": _f8(eT1),
            "edc1": _bf(np.ascontiguousarray(edc1.reshape(L1 // 128, 128).T)),
            "srcw": np.ascontiguousarray(
                np.tile(sidx.reshape(L1 // 16, 16).T, (8, 1))),
            "xoT": _bf(xpad[c * C:(c + 1) * C].T),
        })

    meta = dict(
        L0=L0, meta0=list(zip(cw0.tolist(), cf0.tolist(), cl0.tolist())),
        L1=L1, meta1=list(zip(cw1.tolist(), cf1.tolist(), cl1.tolist(),
                              cs1.tolist())),
        ep_start=[int(v) for v in sub_start1],
        ep_end=[int(sub_start1[s] + sub_len1[s] + sub_pad1[s])
                for s in range(NEP)],
    )
    return shared, per_core, meta


def _build(meta):
    """Build the SPMD Bass program (identical for all 8 cores)."""
    L0, meta0 = meta["L0"], meta["meta0"]
    L1, meta1 = meta["L1"], meta["meta1"]
    ep_start, ep_end = meta["ep_start"], meta["ep_end"]

    nc = bacc.Bacc("TRN2", target_bir_lowering=False, debug=False,
                   num_devices=NC, num_swdge_queues=4,
                   dynamic_dma_scratch_size=int(os.environ.get("KERNEL_DDS", "49152")))
    GELU = (mybir.ActivationFunctionType.Identity
            if os.environ.get("KERNEL_SIM_IDENTITY") == "1"
            else mybir.ActivationFunctionType.Gelu_apprx_tanh)
    EQ = mybir.AluOpType.is_equal

    # I/O
    t_xoT = nc.dram_tensor("xoT", [128, C], BF, kind="ExternalInput")
    t_eT0 = nc.dram_tensor("eT0", [66, L0], F8, kind="ExternalInput")
    t_xeT = nc.dram_tensor("xeT", [128, L0], F8, kind="ExternalInput")
    t_edc0 = nc.dram_tensor("edc0", [128, L0 // 128], BF, kind="ExternalInput")
    t_eT1 = nc.dram_tensor("eT1", [66, L1], F8, kind="ExternalInput")
    t_edc1 = nc.dram_tensor("edc1", [128, L1 // 128], BF, kind="ExternalInput")
    t_srcw = nc.dram_tensor("srcw", [128, L1 // 16], I16, kind="ExternalInput")
    wts = {}
    for nm, shape, dt in [
        ("W1", [128, 128], BF), ("W1f8", [128, 128], F8),
        ("Ws1", [128, 128], BF), ("Wd1", [128, 128], BF),
        ("We1", [66, 128], F8), ("Ws2", [128, 128], BF), ("Wd2", [128, 128], BF),
        ("We2", [66, 128], F8), ("W3", [128, 128], BF),
        ("b1c", [128, 1], F32), ("b3m", [128, 128], F32), ("iota", [128, 128], BF),
        ("identb", [128, 128], BF),
    ]:
        wts[nm] = nc.dram_tensor(nm, shape, dt, kind="ExternalInput")
    t_out = nc.dram_tensor("out", [C, 128], F32, kind="ExternalOutput")

    with tile.TileContext(nc) as tc:
        with (
            tc.tile_pool(name="persist", bufs=1) as pp,
            tc.tile_pool(name="dram", bufs=1, space="DRAM") as dram,
        ):
            wt = {}
            for nm in ["W1", "W1f8", "Ws1", "Wd1", "We1", "Ws2", "Wd2", "We2",
                       "W3", "b1c", "b3m", "iota", "identb"]:
                shape = wts[nm].shape
                dt = {"b1c": F32, "b3m": F32, "W1f8": F8, "We1": F8,
                      "We2": F8}.get(nm, BF)
                wt[nm] = pp.tile(list(shape), dt, tag=f"w_{nm}", name=f"w_{nm}")
                nc.sync.dma_start(out=wt[nm][:], in_=wts[nm][:])
            edc0_t = pp.tile([128, L0 // 128], BF, tag="edc0")
            nc.sync.dma_start(out=edc0_t[:], in_=t_edc0[:])
            edc1_t = pp.tile([128, L1 // 128], BF, tag="edc1")
            nc.sync.dma_start(out=edc1_t[:], in_=t_edc1[:])
            srcw_t = pp.tile([128, L1 // 16], I16, tag="srcw")
            nc.sync.dma_start(out=srcw_t[:], in_=t_srcw[:])
            h_own = pp.tile([128, C], BF, tag="h_own")
            xd_sb = pp.tile([128, C], BF, tag="xd_sb")
            agg_sb = pp.tile([128, C], F32, tag="agg_sb")

            # per-epoch allgather buffers
            ag_in = [dram.tile([EPW[g] * 128, 128], BF, tag=f"agi{g}",
                               name=f"agi{g}") for g in range(NEP)]
            ag_out = [dram.tile([EPW[g] * 128 * NC, 128], BF, tag=f"ago{g}",
                                name=f"ago{g}", addr_space="Shared")
                      for g in range(NEP)]

            # ---------------- dense phase (own nodes only) ----------------
            def dense_own():
                with (
                    tc.tile_pool(name="dB", bufs=3) as dp,
                    tc.tile_pool(name="dBp", bufs=2, space="PSUM") as dq,
                ):
                    ngrp = (C + 511) // 512
                    for gi in range(ngrp):
                        c0 = gi * 512
                        cn = min(512, C - c0)
                        xt = dp.tile([128, 512], BF, tag="xt")
                        nc.sync.dma_start(out=xt[:, :cn], in_=t_xoT[:, c0:c0 + cn])
                        ps = dq.tile([128, 512], F32, tag="ps")
                        nc.tensor.matmul(out=ps[:, :cn], lhsT=wt["W1"][:],
                                         rhs=xt[:, :cn], start=True, stop=True)
                        hT = dp.tile([128, 512], BF, tag="hT")
                        nc.scalar.activation(out=hT[:, :cn], in_=ps[:, :cn],
                                             func=GELU, bias=wt["b1c"][:])
                        for j in range(cn // 128):
                            lw = c0 + j * 128
                            sl = hT[:, j * 128:(j + 1) * 128]
                            pn = dq.tile([128, 128], F32, tag="pn")
                            nc.tensor.matmul(out=pn[:], lhsT=sl, rhs=wt["identb"][:],
                                             start=True, stop=True)
                            nc.vector.tensor_copy(out=h_own[:, lw:lw + 128], in_=pn[:])
                            pd = dq.tile([128, 128], F32, tag="pd")
                            nc.tensor.matmul(out=pd[:], lhsT=sl, rhs=wt["Wd1"][:],
                                             start=True, stop=True)
                            nc.vector.tensor_copy(out=xd_sb[:, lw:lw + 128],
                                                  in_=pd[:])

            # ---------------- merged edge phases ----------------
            def edge_phases(run_l1):
                xsgb = int(os.environ.get("KERNEL_XSGB", "26"))
                with (
                    tc.tile_pool(name="eS", bufs=6) as ep,
                    tc.tile_pool(name="eG", bufs=xsgb) as gp,
                    tc.tile_pool(name="ePH", bufs=1, space="PSUM") as qh,
                    tc.tile_pool(name="ePX", bufs=3, space="PSUM") as qx,
                    tc.tile_pool(name="ePT", bufs=1, space="PSUM") as qt,
                    tc.tile_pool(name="ePA", bufs=2, space="PSUM") as qa,
                    tc.tile_pool(name="eF", bufs=3) as fp,
                ):
                    active_agg = {0: None, 1: None}
                    partial = [False] * WCNT
                    ag_issued = [False] * NEP
                    ag_at = [0] * NEP
                    l0_prog = [0]
                    ag_lead = int(os.environ.get("KERNEL_AGLEAD", "24"))
                    w_done = [-1]
                    r_gop = {k: nc.gpsimd.to_reg(GRP * k) for k in range(1, 12)}
                    xsg_tiles = {}
                    l0blk = [None]

                    def load_l0_block(g0):
                        # one DMA pair covers 4 consecutive L0 groups
                        eTt4 = ep.tile([66, 4 * GRP], F8, tag="eTt4", bufs=3)
                        nc.sync.dma_start(out=eTt4[:],
                                          in_=t_eT0[:, g0:g0 + 4 * GRP])
                        xet4 = ep.tile([128, 4 * GRP], F8, tag="xet4", bufs=3)
                        nc.sync.dma_start(out=xet4[:],
                                          in_=t_xeT[:, g0:g0 + 4 * GRP])
                        l0blk[0] = dict(g0=g0, eTt=eTt4, xet=xet4)

                    def finalize0(w, agg_ps):
                        ws = slice(w * 128, (w + 1) * 128)
                        h1w = fp.tile([128, 128], BF, tag="fh")
                        nc.vector.tensor_add(out=h1w[:], in0=agg_ps[:],
                                             in1=h_own[:, ws])
                        nc.vector.tensor_copy(out=h_own[:, ws], in_=h1w[:])
                        ptr = qt.tile([128, 128], F32, tag="ftp", bufs=1,
                                      name="fptr")
                        nc.tensor.matmul(out=ptr[:], lhsT=h1w[:],
                                         rhs=wt["identb"][:], start=True, stop=True)
                        h1T = fp.tile([128, 128], BF, tag="fh1T")
                        nc.scalar.activation(
                            out=h1T[:], in_=ptr[:],
                            func=mybir.ActivationFunctionType.Identity)
                        pxd = qt.tile([128, 128], F32, tag="ftp", bufs=1,
                                      name="fpxd")
                        nc.tensor.matmul(out=pxd[:], lhsT=h1T[:], rhs=wt["Wd2"][:],
                                         start=True, stop=True)
                        nc.scalar.activation(
                            out=xd_sb[:, ws], in_=pxd[:],
                            func=mybir.ActivationFunctionType.Identity)
                        pxl = qt.tile([128, 128], F32, tag="ftp", bufs=1,
                                      name="fpxl")
                        nc.tensor.matmul(out=pxl[:], lhsT=h1T[:], rhs=wt["Ws2"][:],
                                         start=True, stop=True)
                        xsl = fp.tile([128, 128], BF, tag="fxsl")
                        nc.scalar.activation(
                            out=xsl[:], in_=pxl[:],
                            func=mybir.ActivationFunctionType.Identity)
                        g = next(gg for gg in range(NEP - 1, -1, -1)
                                 if w >= EPW0[gg])
                        lw = w - EPW0[g]
                        nc.sync.dma_start(
                            out=ag_in[g][lw * 128:(lw + 1) * 128, :], in_=xsl[:])
                        if w == EPW0[g] + EPW[g] - 1:
                            nc.gpsimd.collective_compute(
                                "AllGather", mybir.AluOpType.bypass,
                                replica_groups=[list(range(NC))],
                                ins=[ag_in[g][:].opt()],
                                outs=[ag_out[g][:].opt()])
                            ag_issued[g] = True
                            ag_at[g] = l0_prog[0]
                        w_done[0] = w

                    chunks_per_w = [0] * WCNT
                    for cc in range(L1 // 128):
                        if meta1[cc][0] >= 0:
                            chunks_per_w[meta1[cc][0]] += 1
                    chunk_cnt = [0] * WCNT

                    def finalize1_seg(w, last_ep, agg_ps):
                        ws = slice(w * 128, (w + 1) * 128)
                        if not last_ep:
                            if partial[w]:
                                nc.vector.tensor_add(out=agg_sb[:, ws],
                                                     in0=agg_ps[:],
                                                     in1=agg_sb[:, ws])
                            else:
                                nc.vector.tensor_copy(out=agg_sb[:, ws],
                                                      in_=agg_ps[:])
                                partial[w] = True
                            return
                        t1 = fp.tile([128, 128], F32, tag="f1")
                        if partial[w]:
                            nc.vector.tensor_add(out=t1[:], in0=agg_ps[:],
                                                 in1=agg_sb[:, ws])
                        else:
                            nc.vector.tensor_copy(out=t1[:], in_=agg_ps[:])
                        h2w = fp.tile([128, 128], BF, tag="fh2")
                        nc.vector.tensor_add(out=h2w[:], in0=t1[:],
                                             in1=h_own[:, ws])
                        ptr = qt.tile([128, 128], F32, tag="ftp", bufs=1,
                                      name="fptr2")
                        nc.tensor.matmul(out=ptr[:], lhsT=h2w[:],
                                         rhs=wt["identb"][:], start=True, stop=True)
                        h2T = fp.tile([128, 128], BF, tag="fh2T")
                        nc.vector.tensor_copy(out=h2T[:], in_=ptr[:])
                        po = qt.tile([128, 128], F32, tag="ftp", bufs=1, name="fpo")
                        nc.tensor.matmul(out=po[:], lhsT=h2T[:], rhs=wt["W3"][:],
                                         start=True, stop=True)
                        osb = fp.tile([128, 128], F32, tag="fosb")
                        nc.vector.tensor_add(out=osb[:], in0=po[:], in1=wt["b3m"][:])
                        nc.sync.dma_start(out=t_out[w * 128:(w + 1) * 128, :],
                                          in_=osb[:])

                    qn = [0]

                    def emit_gather(u):
                        # one gather op for the whole unit (contiguous stream
                        # range); groups consume slices of the tile.
                        g0 = u["g0s"][0]
                        k = len(u["g0s"])
                        n_idx = GRP * k
                        ep_i = next(gg for gg in range(NEP)
                                    if ep_start[gg] <= g0 < ep_end[gg])
                        xsg = gp.tile([128, 4 * k, 128], BF, tag=f"xsg{k}",
                                      bufs=(xsgb if k <= 4 else 2))
                        nc.gpsimd.dma_gather(
                            xsg[:], ag_out[ep_i][:],
                            srcw_t[:, g0 // 16:(g0 + n_idx) // 16],
                            n_idx, r_gop[k], 128, elem_step=128,
                            queue_num=qn[0] % 4)
                        qn[0] += 1
                        for gg0 in u["g0s"]:
                            xsg_tiles[gg0 // GRP] = (xsg, (gg0 - g0) // 128)

                    def emit_group(layer, g0, ufirst=False, ulast=False):
                        We = wt["We1"] if layer == 0 else wt["We2"]
                        meta_ = meta0 if layer == 0 else meta1
                        edc_t = edc0_t if layer == 0 else edc1_t
                        t_eT = t_eT0 if layer == 0 else t_eT1
                        cc0 = g0 // 128
                        chunks = [meta_[cc0 + j] for j in range(4)]
                        if all(ch[0] < 0 for ch in chunks):
                            return
                        jlast = max(j for j in range(4) if chunks[j][0] >= 0)
                        if layer == 1:
                            xsg, xoff = xsg_tiles.pop(g0 // GRP)

                        if layer == 0:
                            blk = l0blk[0]
                            eTt = blk["eTt"]
                            xet = blk["xet"]
                            boff = g0 - blk["g0"]
                        else:
                            eTt = ep.tile([66, GRP], F8, tag="eTt")
                            nc.sync.dma_start(out=eTt[:],
                                              in_=t_eT[:, g0:g0 + GRP])
                            boff = 0

                        P4 = ep.tile([128, 4, 128], BF, tag="P4")
                        nc.vector.tensor_tensor(
                            out=P4[:],
                            in0=wt["iota"][:, None, :].to_broadcast([128, 4, 128]),
                            in1=edc_t[:, cc0:cc0 + 4]
                                .to_broadcast([128, 4, 128]),
                            op=EQ)
                        ptp = qt.tile([128, 4, 128], F32, tag="ptp", bufs=1,
                                      name="ptp")
                        for j in range(4):
                            nc.tensor.matmul(out=ptp[:, j, :], lhsT=P4[:, j, :],
                                             rhs=wt["identb"][:],
                                             start=True, stop=True)
                        Pt = ep.tile([128, 4, 128], BF, tag="Pt")
                        if layer == 0:
                            nc.scalar.activation(
                                out=Pt[:], in_=ptp[:],
                                func=mybir.ActivationFunctionType.Identity)
                        else:
                            nc.vector.tensor_copy(out=Pt[:], in_=ptp[:])

                        # edge-major pre-activation accumulation [edge, (j, feat)].
                        # PSUM accumulation-group rule: exactly ONE start=True
                        # per tile (it resets the whole bank); everything else
                        # accumulates, the final xd matmuls carry stop.
                        pxs = qx.tile([128, 4, 128], F32, tag="pxs")
                        for j in range(4):
                            o_ = boff + j * 128
                            nc.tensor.matmul(out=pxs[:, j, :],
                                             lhsT=eTt[:, o_:o_ + 128],
                                             rhs=We[:], start=(j == 0),
                                             stop=False, skip_group_check=True)
                        if layer == 0:
                            ph = qh.tile([128, GRP], F32, tag="ph")
                            nc.tensor.matmul(out=ph[:], lhsT=wt["W1f8"][:],
                                             rhs=xet[:, boff:boff + GRP],
                                             start=True, stop=True)
                            heT = ep.tile([128, GRP], BF, tag="heT")
                            nc.scalar.activation(out=heT[:], in_=ph[:],
                                                 func=GELU, bias=wt["b1c"][:])
                            for j in range(4):
                                nc.tensor.matmul(
                                    out=pxs[:, j, :],
                                    lhsT=heT[:, j * 128:(j + 1) * 128],
                                    rhs=wt["Ws1"][:], start=False, stop=False)
                        else:
                            nc.tensor.matmul(out=pxs[:], lhsT=wt["identb"][:],
                                             rhs=xsg[:, xoff:xoff + 4, :],
                                             start=False, stop=False)
                        for j in range(4):
                            w_j = chunks[j][0]
                            if w_j < 0:
                                w_j = 0  # pad chunk: P4 row is zero anyway
                            nc.tensor.matmul(
                                out=pxs[:, j, :], lhsT=Pt[:, j, :],
                                rhs=xd_sb[:, w_j * 128:(w_j + 1) * 128],
                                start=False, stop=True,
                                skip_group_check=True)

                        mg = ep.tile([128, 4, 128], BF, tag="mg")
                        nc.scalar.activation(out=mg[:], in_=pxs[:], func=GELU)

                        for j in range(4):
                            ch = meta_[cc0 + j]
                            w = ch[0]
                            if w < 0:
                                continue
                            first, last = ch[1], ch[2]
                            if layer == 1:
                                # unit-relative overrides: a segment split
                                # across units flushes via agg_sb partials
                                if ufirst and j == 0:
                                    first = True
                                if ulast and j == jlast:
                                    last = True
                                chunk_cnt[w] += 1
                            if first:
                                active_agg[layer] = qa.tile(
                                    [128, 128], F32, tag=f"agg{layer}",
                                    name=f"agg_ps{layer}", bufs=1)
                            nc.tensor.matmul(out=active_agg[layer][:],
                                             lhsT=P4[:, j, :], rhs=mg[:, j, :],
                                             start=first, stop=last)
                            if last:
                                if layer == 0:
                                    finalize0(w, active_agg[layer])
                                else:
                                    finalize1_seg(
                                        w, chunk_cnt[w] == chunks_per_w[w],
                                        active_agg[layer])

                    # ---- L1 schedule: fixed ~4-group units (one gather op
                    # each), drained OUT OF ORDER as (epoch AG done) x (dst
                    # window finalized) eligibility allows -- avoids
                    # head-of-line blocking of later epochs behind an early
                    # epoch's high windows. Segments split across unit
                    # boundaries flush through agg_sb partials.
                    units = []          # list of dicts: g0s, maxw, ep
                    if run_l1:
                        gl = []
                        for g0 in range(0, L1, GRP):
                            cc0 = g0 // 128
                            chs = [meta1[cc0 + j] for j in range(4)]
                            ws_ = [ch[0] for ch in chs if ch[0] >= 0]
                            if not ws_:
                                continue
                            ep_i = next(gg for gg in range(NEP)
                                        if ep_start[gg] <= g0 < ep_end[gg])
                            gl.append((g0, ep_i, max(ws_)))
                        i = 0
                        while i < len(gl):
                            g0s = [gl[i][0]]
                            ep_i, maxw = gl[i][1], gl[i][2]
                            j = i + 1
                            ucap = int(os.environ.get("KERNEL_UCAP", "1"))
                            while (j < len(gl) and len(g0s) < ucap
                                   and gl[j][1] == ep_i
                                   and gl[j][0] == g0s[-1] + GRP):
                                g0s.append(gl[j][0])
                                maxw = max(maxw, gl[j][2])
                                j += 1
                            units.append(dict(g0s=g0s, maxw=maxw, ep=ep_i))
                            i = j
                    n_units = len(units)
                    done_u = [False] * n_units
                    n_drained = [0]
                    gathered = []       # FIFO of units with gathers emitted

                    def select_unit():
                        for ui in range(n_units):
                            if done_u[ui]:
                                continue
                            u = units[ui]
                            if not ag_issued[u["ep"]]:
                                continue
                            if l0_prog[0] - ag_at[u["ep"]] < ag_lead:
                                continue
                            if u["maxw"] > w_done[0]:
                                continue
                            done_u[ui] = True
                            return u
                        return None

                    gahead = int(os.environ.get("KERNEL_GAHEAD", "24"))

                    glag = int(os.environ.get("KERNEL_GLAG", "3"))

                    def drain_gathers():
                        while len(gathered) < gahead:
                            u = select_unit()
                            if u is None:
                                return
                            emit_gather(u)
                            gathered.append((u, l0_prog[0]))

                    def drain_l1(cap=2, final=False):
                        done = 0
                        while gathered and done < cap:
                            u, tick = gathered[0]
                            # wait for the unit's gather DMA to have had time
                            # to land before the PE consumes it
                            if not final and l0_prog[0] - tick < glag:
                                return
                            gathered.pop(0)
                            for i_, g0 in enumerate(u["g0s"]):
                                emit_group(1, g0, ufirst=(i_ == 0),
                                           ulast=(i_ == len(u["g0s"]) - 1))
                            n_drained[0] += 1
                            done += 1

                    for g0 in range(0, L0, GRP):
                        if g0 % (4 * GRP) == 0:
                            load_l0_block(g0)
                        emit_group(0, g0)
                        l0_prog[0] += 1
                        if run_l1:
                            drain_gathers()
                            drain_l1()
                    if run_l1:
                        w_done[0] = WCNT  # everything finalized
                        guard = 0
                        while n_drained[0] < n_units:
                            drain_gathers()
                            drain_l1(final=True)
                            l0_prog[0] += 1  # advance virtual time for ag_lead
                            guard += 1
                            assert guard < 100000, "final drain stuck"
                        assert n_drained[0] == n_units

            # ---------------- program ----------------
            phases = int(os.environ.get("KERNEL_PHASES", "3"))
            if phases >= 2:
                dense_own()
                edge_phases(run_l1=(phases >= 3))
            else:
                dense_own()
            if phases < 3:
                with tc.tile_pool(name="dbg", bufs=2) as dbp:
                    for w in range(WCNT):
                        dsb = dbp.tile([128, 128], F32, tag="dsb")
                        nc.vector.tensor_copy(
                            out=dsb[:], in_=h_own[:, w * 128:(w + 1) * 128])
                        nc.sync.dma_start(
                            out=t_out[w * 128:(w + 1) * 128, :], in_=dsb[:])

    nc.finalize()
    return nc


_CACHE = {}


def _get_program(meta):
    key = (meta["L0"], meta["L1"], tuple(meta["meta0"]), tuple(meta["meta1"]),
           tuple(meta["ep_start"]), tuple(meta["ep_end"]))
    if key not in _CACHE:
        _CACHE[key] = _build(meta)
    return _CACHE[key]


def kernel(**inputs):
    shared, per_core, meta = _prep(inputs)
    nc = _get_program(meta)
    in_maps = []
    for c in range(NC):
        m = dict(shared)
        m.update(per_core[c])
        in_maps.append(m)
    trace = os.environ.get("KERNEL_TRACE", "0") == "1"
    kw = {}
    if trace:
        kw = dict(trace=True, trace_kwargs={"title": "gnn_mp_v3"})
    res = run_bass_kernel_spmd(nc, in_maps, core_ids=list(range(NC)), **kw)
    if trace and res.exec_time_ns is not None:
        print(f"HW exec time: {res.exec_time_ns} ns")
        if res.instructions_and_trace:
            print("trace:", res.instructions_and_trace[1])
    out = np.concatenate([res.results[c]["out"] for c in range(NC)], axis=0)
    return np.ascontiguousarray(out[:N]).astype(np.float32)


# revision 41
# speedup vs baseline: 1.1244x; 1.1244x over previous
# Trainium2 Bass kernel for the 2-layer GNN message-passing block.
# Self-contained: hardcodes shapes; takes full inputs, shards across 8 cores,
# returns the full [50000, 128] float32 output.
#
# Design (v3):
#  - Layer 0: no gather. Host streams x[src_e] feature-major (xeT); the device
#    computes GELU(W1^T x_e + b1) per edge, then accumulates the Ws1/We1/xd
#    terms into an EDGE-major PSUM (partition=edge) so the message GELU writes
#    scatter-ready SBUF directly (no PE transpose, single scalar op).
#  - Biases (mp b) are folded into an extra all-ones row on the edge-feature
#    stream: eT is [65, L], We' = [We; b].
#  - Layer 1: dma_gather (node-major rows) of xs1 from per-epoch AllGather
#    buffers at GOP=GRP=512 (one gather per compute group), consumed by a
#    single identity matmul into the edge-major PSUM. 6 front-loaded epochs
#    start the gather DMA early; gather emission is paced to keep GpSimd
#    responsive for AllGather triggers.
#  - Scatter per chunk via one-hot matmul into PSUM; Pt (the one-hot used for
#    the xd gather) is the PE-transpose of P4.
import os
import sys

sys.path.insert(0, "/opt/trn_rl_repo")

import numpy as np
import ml_dtypes

import concourse.bacc as bacc
import concourse.tile as tile
from concourse import mybir
from concourse.bass_utils import run_bass_kernel_spmd

BF16 = ml_dtypes.bfloat16
FP8 = ml_dtypes.float8_e4m3

N = 50000
NPAD = 50176
NC = 8
C = NPAD // NC            # 6272 nodes per core
WCNT = C // 128           # 49 windows of 128 nodes
NEP = 4                   # epochs (groups of local windows) for layer-1
EPW = [10, 18, 17, 4]     # windows per epoch (small first -> early AG)
EPW0 = [0, 10, 28, 45]
GRP = 512                 # edges per compute group (4 chunks) == gather op

F32 = mybir.dt.float32
BF = mybir.dt.bfloat16
F8 = mybir.dt.float8e4
I16 = mybir.dt.int16


def _bf(x):
    return np.ascontiguousarray(x.astype(BF16))


def _f8(x):
    return np.ascontiguousarray(np.asarray(x, np.float32).astype(FP8))


def _sort_stream(dst_local, sub, nsub, owner, extra_pad_unit):
    """Shared-layout edge stream sort.

    Edges keyed by (owner, sub, win). Returns per-core segment layout shared
    across cores (max counts), satisfying: each (sub, win) segment is a
    multiple of 128 edges (>=128), and each sub block is a multiple of
    extra_pad_unit edges.
    Returns (order, dest, seg info, L, chunk metadata arrays).
    """
    E = dst_local.shape[0]
    win = dst_local // 128
    key = (owner * nsub + sub) * WCNT + win
    order = np.argsort(key, kind="stable")
    ksort = key[order]
    counts_flat = np.bincount(key, minlength=NC * nsub * WCNT)
    counts = counts_flat.reshape(NC, nsub, WCNT)

    nch = np.maximum(1, -(-counts.max(axis=0) // 128)).astype(np.int64)  # [nsub, WCNT]
    seg_len = nch * 128
    sub_len = seg_len.sum(axis=1)                      # [nsub]
    sub_pad = (-sub_len) % extra_pad_unit
    sub_start = np.zeros(nsub, np.int64)
    pos = 0
    for s in range(nsub):
        sub_start[s] = pos
        pos += sub_len[s] + sub_pad[s]
    L = int(pos)

    seg_start = np.zeros((nsub, WCNT), np.int64)
    for s in range(nsub):
        p = sub_start[s]
        for w in range(WCNT):
            seg_start[s, w] = p
            p += seg_len[s, w]

    group_first = np.cumsum(counts_flat) - counts_flat
    within = np.arange(E, dtype=np.int64) - group_first[ksort]
    dest = seg_start[sub[order], win[order]] + within

    nchunks = L // 128
    cw = np.full(nchunks, -1, np.int64)     # window (-1 = pad chunk)
    cf = np.zeros(nchunks, bool)            # first chunk of segment
    cl = np.zeros(nchunks, bool)            # last chunk of segment
    cs = np.zeros(nchunks, np.int64)        # sub index
    for s in range(nsub):
        for w in range(WCNT):
            a = int(seg_start[s, w]) // 128
            n = int(nch[s, w])
            cw[a:a + n] = w
            cf[a] = True
            cl[a + n - 1] = True
            cs[a:a + n] = s
        pe = (int(sub_start[s]) + int(sub_len[s])) // 128
        pe2 = pe + int(sub_pad[s]) // 128
        cs[pe:pe2] = s
    return order, dest, L, cw, cf, cl, cs, sub_start, sub_len, sub_pad


def _prep(inputs):
    """Host-side graph partitioning / stream layout. Index+layout prep only."""
    src = np.asarray(inputs["edge_index"][0]).astype(np.int64)
    dst = np.asarray(inputs["edge_index"][1]).astype(np.int64)
    ef = np.asarray(inputs["edge_features"]).astype(np.float32)

    d_owner = dst // C
    dl = dst - d_owner * C

    s_owner = src // C
    s_lw = (src - s_owner * C) // 128
    s_ep = np.searchsorted(np.array(EPW0), s_lw, side="right") - 1

    # ---- layer-0 stream: sorted by (dst window) only ----
    z = np.zeros_like(dst)
    (o0, de0, L0, cw0, cf0, cl0, _, _, _, _) = _sort_stream(
        dl, z, 1, d_owner, 4 * GRP)

    # ---- layer-1 stream: sorted by (src epoch, dst window) ----
    (o1, de1, L1, cw1, cf1, cl1, cs1, sub_start1, sub_len1, sub_pad1) = \
        _sort_stream(dl, s_ep, NEP, d_owner, GRP)

    # epoch-buffer row index for every edge (gather idx within its epoch buf)
    ep_rows = np.array([EPW[g] * 128 for g in range(NEP)])
    g = s_ep
    ep0 = np.array(EPW0)[g]
    row = s_owner * ep_rows[g] + (s_lw - ep0) * 128 + (src - s_owner * C - s_lw * 128)
    assert row.max() < 32768

    x = np.asarray(inputs["x"]).astype(np.float32)
    xpad = np.zeros((NPAD, 128), np.float32)
    xpad[:N] = x
    x_bf = xpad.astype(BF16)

    iota = np.tile(np.arange(128, dtype=np.float32)[None, :], (128, 1))
    ident = np.eye(128, dtype=np.float32)

    def _we_bias(w, b):
        # [66, 128]: We with the bias folded in as an extra contraction row
        # (plus a zero row to keep the contraction partition count even)
        return _f8(np.vstack([np.asarray(w, np.float32),
                              np.asarray(b, np.float32)[None, :],
                              np.zeros((1, 128), np.float32)]))

    shared = {
        "W1": _bf(np.asarray(inputs["ff1_W"], np.float32)),
        "W1f8": _f8(np.asarray(inputs["ff1_W"], np.float32)),
        "Ws1": _bf(np.asarray(inputs["mp1_Wsrc"], np.float32)),
        "Wd1": _bf(np.asarray(inputs["mp1_Wdst"], np.float32)),
        "We1": _we_bias(inputs["mp1_We"], inputs["mp1_b"]),
        "Ws2": _bf(np.asarray(inputs["mp2_Wsrc"], np.float32)),
        "Wd2": _bf(np.asarray(inputs["mp2_Wdst"], np.float32)),
        "We2": _we_bias(inputs["mp2_We"], inputs["mp2_b"]),
        "W3": _bf(np.asarray(inputs["ff2_W"], np.float32)),
        "b1c": np.ascontiguousarray(
            np.asarray(inputs["ff1_b"], np.float32)[:, None]),
        "b3m": np.ascontiguousarray(
            np.tile(np.asarray(inputs["ff2_b"], np.float32)[None, :], (128, 1))),
        "iota": _bf(iota),
        "identb": _bf(ident),
    }

    per_core = []
    for c in range(NC):
        # layer-0 per-core stream (edge features + ones row for the bias)
        m0 = d_owner[o0] == c
        e0_ids = o0[m0]
        dp0 = de0[m0]
        eT0 = np.zeros((66, L0), np.float32)
        eT0[:64, dp0] = ef[e0_ids].T
        eT0[64, dp0] = 1.0
        xeT = np.zeros((128, L0), FP8)
        xeT[:, dp0] = xpad[src[e0_ids]].astype(FP8).T
        edc0 = np.full(L0, -1.0, np.float32)
        edc0[dp0] = dl[e0_ids] % 128

        # layer-1 per-core stream
        m1 = d_owner[o1] == c
        e1_ids = o1[m1]
        dp1 = de1[m1]
        eT1 = np.zeros((66, L1), np.float32)
        eT1[:64, dp1] = ef[e1_ids].T
        eT1[64, dp1] = 1.0
        edc1 = np.full(L1, -1.0, np.float32)
        edc1[dp1] = dl[e1_ids] % 128
        sidx = np.zeros(L1, np.int16)
        sidx[dp1] = row[e1_ids].astype(np.int16)

        per_core.append({
            "eT0": _f8(eT0),
            "xeT": np.ascontiguousarray(xeT),
            "edc0": _bf(np.ascontiguousarray(edc0.reshape(L0 // 128, 128).T)),
            "eT1": _f8(eT1),
            "edc1": _bf(np.ascontiguousarray(edc1.reshape(L1 // 128, 128).T)),
            "srcw": np.ascontiguousarray(
                np.tile(sidx.reshape(L1 // 16, 16).T, (8, 1))),
            "xoT": _bf(xpad[c * C:(c + 1) * C].T),
        })

    meta = dict(
        L0=L0, meta0=list(zip(cw0.tolist(), cf0.tolist(), cl0.tolist())),
        L1=L1, meta1=list(zip(cw1.tolist(), cf1.tolist(), cl1.tolist(),
                              cs1.tolist())),
        ep_start=[int(v) for v in sub_start1],
        ep_end=[int(sub_start1[s] + sub_len1[s] + sub_pad1[s])
                for s in range(NEP)],
    )
    return shared, per_core, meta


def _build(meta):
    """Build the SPMD Bass program (identical for all 8 cores)."""
    L0, meta0 = meta["L0"], meta["meta0"]
    L1, meta1 = meta["L1"], meta["meta1"]
    ep_start, ep_end = meta["ep_start"], meta["ep_end"]

    nc = bacc.Bacc("TRN2", target_bir_lowering=False, debug=False,
                   num_devices=NC, num_swdge_queues=4,
                   dynamic_dma_scratch_size=int(os.environ.get("KERNEL_DDS", "49152")))
    GELU = (mybir.ActivationFunctionType.Identity
            if os.environ.get("KERNEL_SIM_IDENTITY") == "1"
            else mybir.ActivationFunctionType.Gelu_apprx_tanh)
    EQ = mybir.AluOpType.is_equal

    # I/O
    t_xoT = nc.dram_tensor("xoT", [128, C], BF, kind="ExternalInput")
    t_eT0 = nc.dram_tensor("eT0", [66, L0], F8, kind="ExternalInput")
    t_xeT = nc.dram_tensor("xeT", [128, L0], F8, kind="ExternalInput")
    t_edc0 = nc.dram_tensor("edc0", [128, L0 // 128], BF, kind="ExternalInput")
    t_eT1 = nc.dram_tensor("eT1", [66, L1], F8, kind="ExternalInput")
    t_edc1 = nc.dram_tensor("edc1", [128, L1 // 128], BF, kind="ExternalInput")
    t_srcw = nc.dram_tensor("srcw", [128, L1 // 16], I16, kind="ExternalInput")
    wts = {}
    for nm, shape, dt in [
        ("W1", [128, 128], BF), ("W1f8", [128, 128], F8),
        ("Ws1", [128, 128], BF), ("Wd1", [128, 128], BF),
        ("We1", [66, 128], F8), ("Ws2", [128, 128], BF), ("Wd2", [128, 128], BF),
        ("We2", [66, 128], F8), ("W3", [128, 128], BF),
        ("b1c", [128, 1], F32), ("b3m", [128, 128], F32), ("iota", [128, 128], BF),
        ("identb", [128, 128], BF),
    ]:
        wts[nm] = nc.dram_tensor(nm, shape, dt, kind="ExternalInput")
    t_out = nc.dram_tensor("out", [C, 128], F32, kind="ExternalOutput")

    with tile.TileContext(nc) as tc:
        with (
            tc.tile_pool(name="persist", bufs=1) as pp,
            tc.tile_pool(name="dram", bufs=1, space="DRAM") as dram,
        ):
            wt = {}
            for nm in ["W1", "W1f8", "Ws1", "Wd1", "We1", "Ws2", "Wd2", "We2",
                       "W3", "b1c", "b3m", "iota", "identb"]:
                shape = wts[nm].shape
                dt = {"b1c": F32, "b3m": F32, "W1f8": F8, "We1": F8,
                      "We2": F8}.get(nm, BF)
                wt[nm] = pp.tile(list(shape), dt, tag=f"w_{nm}", name=f"w_{nm}")
                nc.sync.dma_start(out=wt[nm][:], in_=wts[nm][:])
            edc0_t = pp.tile([128, L0 // 128], BF, tag="edc0")
            nc.sync.dma_start(out=edc0_t[:], in_=t_edc0[:])
            edc1_t = pp.tile([128, L1 // 128], BF, tag="edc1")
            nc.sync.dma_start(out=edc1_t[:], in_=t_edc1[:])
            srcw_t = pp.tile([128, L1 // 16], I16, tag="srcw")
            nc.sync.dma_start(out=srcw_t[:], in_=t_srcw[:])
            h_own = pp.tile([128, C], BF, tag="h_own")
            xd_sb = pp.tile([128, C], BF, tag="xd_sb")
            agg_sb = pp.tile([128, C], F32, tag="agg_sb")

            # per-epoch allgather buffers
            ag_in = [dram.tile([EPW[g] * 128, 128], BF, tag=f"agi{g}",
                               name=f"agi{g}") for g in range(NEP)]
            ag_out = [dram.tile([EPW[g] * 128 * NC, 128], BF, tag=f"ago{g}",
                                name=f"ago{g}", addr_space="Shared")
                      for g in range(NEP)]

            # ---------------- dense phase (own nodes only) ----------------
            def dense_own():
                with (
                    tc.tile_pool(name="dB", bufs=3) as dp,
                    tc.tile_pool(name="dBp", bufs=2, space="PSUM") as dq,
                ):
                    ngrp = (C + 511) // 512
                    for gi in range(ngrp):
                        c0 = gi * 512
                        cn = min(512, C - c0)
                        xt = dp.tile([128, 512], BF, tag="xt")
                        nc.sync.dma_start(out=xt[:, :cn], in_=t_xoT[:, c0:c0 + cn])
                        ps = dq.tile([128, 512], F32, tag="ps")
                        nc.tensor.matmul(out=ps[:, :cn], lhsT=wt["W1"][:],
                                         rhs=xt[:, :cn], start=True, stop=True)
                        hT = dp.tile([128, 512], BF, tag="hT")
                        nc.scalar.activation(out=hT[:, :cn], in_=ps[:, :cn],
                                             func=GELU, bias=wt["b1c"][:])
                        for j in range(cn // 128):
                            lw = c0 + j * 128
                            sl = hT[:, j * 128:(j + 1) * 128]
                            pn = dq.tile([128, 128], F32, tag="pn")
                            nc.tensor.matmul(out=pn[:], lhsT=sl, rhs=wt["identb"][:],
                                             start=True, stop=True)
                            nc.vector.tensor_copy(out=h_own[:, lw:lw + 128], in_=pn[:])
                            pd = dq.tile([128, 128], F32, tag="pd")
                            nc.tensor.matmul(out=pd[:], lhsT=sl, rhs=wt["Wd1"][:],
                                             start=True, stop=True)
                            nc.vector.tensor_copy(out=xd_sb[:, lw:lw + 128],
                                                  in_=pd[:])

            # ---------------- merged edge phases ----------------
            def edge_phases(run_l1):
                xsgb = int(os.environ.get("KERNEL_XSGB", "26"))
                with (
                    tc.tile_pool(name="eS", bufs=6) as ep,
                    tc.tile_pool(name="eG", bufs=xsgb) as gp,
                    tc.tile_pool(name="ePH", bufs=1, space="PSUM") as qh,
                    tc.tile_pool(name="ePX", bufs=3, space="PSUM") as qx,
                    tc.tile_pool(name="ePT", bufs=1, space="PSUM") as qt,
                    tc.tile_pool(name="ePA", bufs=2, space="PSUM") as qa,
                    tc.tile_pool(name="eF", bufs=3) as fp,
                ):
                    active_agg = {0: None, 1: None}
                    partial = [False] * WCNT
                    ag_issued = [False] * NEP
                    ag_at = [0] * NEP
                    l0_prog = [0]
                    ag_lead = int(os.environ.get("KERNEL_AGLEAD", "24"))
                    w_done = [-1]
                    r_gop = {k: nc.gpsimd.to_reg(GRP * k) for k in range(1, 12)}
                    xsg_tiles = {}
                    l0blk = [None]

                    def load_l0_block(g0):
                        # one DMA pair covers 4 consecutive L0 groups
                        eTt4 = ep.tile([66, 4 * GRP], F8, tag="eTt4", bufs=3)
                        nc.sync.dma_start(out=eTt4[:],
                                          in_=t_eT0[:, g0:g0 + 4 * GRP])
                        xet4 = ep.tile([128, 4 * GRP], F8, tag="xet4", bufs=3)
                        nc.sync.dma_start(out=xet4[:],
                                          in_=t_xeT[:, g0:g0 + 4 * GRP])
                        l0blk[0] = dict(g0=g0, eTt=eTt4, xet=xet4)

                    def finalize0(w, agg_ps):
                        ws = slice(w * 128, (w + 1) * 128)
                        h1w = fp.tile([128, 128], BF, tag="fh")
                        nc.vector.tensor_add(out=h1w[:], in0=agg_ps[:],
                                             in1=h_own[:, ws])
                        nc.vector.tensor_copy(out=h_own[:, ws], in_=h1w[:])
                        ptr = qt.tile([128, 128], F32, tag="ftp", bufs=1,
                                      name="fptr")
                        nc.tensor.matmul(out=ptr[:], lhsT=h1w[:],
                                         rhs=wt["identb"][:], start=True, stop=True)
                        h1T = fp.tile([128, 128], BF, tag="fh1T")
                        nc.scalar.activation(
                            out=h1T[:], in_=ptr[:],
                            func=mybir.ActivationFunctionType.Identity)
                        pxd = qt.tile([128, 128], F32, tag="ftp", bufs=1,
                                      name="fpxd")
                        nc.tensor.matmul(out=pxd[:], lhsT=h1T[:], rhs=wt["Wd2"][:],
                                         start=True, stop=True)
                        nc.scalar.activation(
                            out=xd_sb[:, ws], in_=pxd[:],
                            func=mybir.ActivationFunctionType.Identity)
                        pxl = qt.tile([128, 128], F32, tag="ftp", bufs=1,
                                      name="fpxl")
                        nc.tensor.matmul(out=pxl[:], lhsT=h1T[:], rhs=wt["Ws2"][:],
                                         start=True, stop=True)
                        xsl = fp.tile([128, 128], BF, tag="fxsl")
                        nc.scalar.activation(
                            out=xsl[:], in_=pxl[:],
                            func=mybir.ActivationFunctionType.Identity)
                        g = next(gg for gg in range(NEP - 1, -1, -1)
                                 if w >= EPW0[gg])
                        lw = w - EPW0[g]
                        nc.sync.dma_start(
                            out=ag_in[g][lw * 128:(lw + 1) * 128, :], in_=xsl[:])
                        if w == EPW0[g] + EPW[g] - 1:
                            nc.gpsimd.collective_compute(
                                "AllGather", mybir.AluOpType.bypass,
                                replica_groups=[list(range(NC))],
                                ins=[ag_in[g][:].opt()],
                                outs=[ag_out[g][:].opt()])
                            ag_issued[g] = True
                            ag_at[g] = l0_prog[0]
                        w_done[0] = w

                    chunks_per_w = [0] * WCNT
                    for cc in range(L1 // 128):
                        if meta1[cc][0] >= 0:
                            chunks_per_w[meta1[cc][0]] += 1
                    chunk_cnt = [0] * WCNT

                    def finalize1_seg(w, last_ep, agg_ps):
                        ws = slice(w * 128, (w + 1) * 128)
                        if not last_ep:
                            if partial[w]:
                                nc.vector.tensor_add(out=agg_sb[:, ws],
                                                     in0=agg_ps[:],
                                                     in1=agg_sb[:, ws])
                            else:
                                nc.vector.tensor_copy(out=agg_sb[:, ws],
                                                      in_=agg_ps[:])
                                partial[w] = True
                            return
                        t1 = fp.tile([128, 128], F32, tag="f1")
                        if partial[w]:
                            nc.vector.tensor_add(out=t1[:], in0=agg_ps[:],
                                                 in1=agg_sb[:, ws])
                        else:
                            nc.vector.tensor_copy(out=t1[:], in_=agg_ps[:])
                        h2w = fp.tile([128, 128], BF, tag="fh2")
                        nc.vector.tensor_add(out=h2w[:], in0=t1[:],
                                             in1=h_own[:, ws])
                        ptr = qt.tile([128, 128], F32, tag="ftp", bufs=1,
                                      name="fptr2")
                        nc.tensor.matmul(out=ptr[:], lhsT=h2w[:],
                                         rhs=wt["identb"][:], start=True, stop=True)
                        h2T = fp.tile([128, 128], BF, tag="fh2T")
                        nc.vector.tensor_copy(out=h2T[:], in_=ptr[:])
                        po = qt.tile([128, 128], F32, tag="ftp", bufs=1, name="fpo")
                        nc.tensor.matmul(out=po[:], lhsT=h2T[:], rhs=wt["W3"][:],
                                         start=True, stop=True)
                        osb = fp.tile([128, 128], F32, tag="fosb")
                        nc.vector.tensor_add(out=osb[:], in0=po[:], in1=wt["b3m"][:])
                        nc.sync.dma_start(out=t_out[w * 128:(w + 1) * 128, :],
                                          in_=osb[:])

                    qn = [0]

                    def emit_gather(u):
                        # one gather op for the whole unit (contiguous stream
                        # range); groups consume slices of the tile.
                        g0 = u["g0s"][0]
                        k = len(u["g0s"])
                        n_idx = GRP * k
                        ep_i = next(gg for gg in range(NEP)
                                    if ep_start[gg] <= g0 < ep_end[gg])
                        xsg = gp.tile([128, 4 * k, 128], BF, tag=f"xsg{k}",
                                      bufs=(xsgb if k <= 4 else 2))
                        nc.gpsimd.dma_gather(
                            xsg[:], ag_out[ep_i][:],
                            srcw_t[:, g0 // 16:(g0 + n_idx) // 16],
                            n_idx, r_gop[k], 128, elem_step=128,
                            queue_num=qn[0] % 4)
                        qn[0] += 1
                        for gg0 in u["g0s"]:
                            xsg_tiles[gg0 // GRP] = (xsg, (gg0 - g0) // 128)

                    def emit_group(layer, g0, ufirst=False, ulast=False):
                        We = wt["We1"] if layer == 0 else wt["We2"]
                        meta_ = meta0 if layer == 0 else meta1
                        edc_t = edc0_t if layer == 0 else edc1_t
                        t_eT = t_eT0 if layer == 0 else t_eT1
                        cc0 = g0 // 128
                        chunks = [meta_[cc0 + j] for j in range(4)]
                        if all(ch[0] < 0 for ch in chunks):
                            return
                        jlast = max(j for j in range(4) if chunks[j][0] >= 0)
                        if layer == 1:
                            xsg, xoff = xsg_tiles.pop(g0 // GRP)

                        if layer == 0:
                            blk = l0blk[0]
                            eTt = blk["eTt"]
                            xet = blk["xet"]
                            boff = g0 - blk["g0"]
                        else:
                            eTt = ep.tile([66, GRP], F8, tag="eTt")
                            nc.sync.dma_start(out=eTt[:],
                                              in_=t_eT[:, g0:g0 + GRP])
                            boff = 0

                        P4 = ep.tile([128, 4, 128], BF, tag="P4")
                        nc.vector.tensor_tensor(
                            out=P4[:],
                            in0=wt["iota"][:, None, :].to_broadcast([128, 4, 128]),
                            in1=edc_t[:, cc0:cc0 + 4]
                                .to_broadcast([128, 4, 128]),
                            op=EQ)
                        ptp = qt.tile([128, 4, 128], F32, tag="ptp", bufs=1,
                                      name="ptp")
                        for j in range(4):
                            nc.tensor.matmul(out=ptp[:, j, :], lhsT=P4[:, j, :],
                                             rhs=wt["identb"][:],
                                             start=True, stop=True)
                        Pt = ep.tile([128, 4, 128], BF, tag="Pt")
                        if layer == 0:
                            nc.scalar.activation(
                                out=Pt[:], in_=ptp[:],
                                func=mybir.ActivationFunctionType.Identity)
                        else:
                            nc.vector.tensor_copy(out=Pt[:], in_=ptp[:])

                        # edge-major pre-activation accumulation [edge, (j, feat)].
                        # PSUM accumulation-group rule: exactly ONE start=True
                        # per tile (it resets the whole bank); everything else
                        # accumulates, the final xd matmuls carry stop.
                        pxs = qx.tile([128, 4, 128], F32, tag="pxs")
                        for j in range(4):
                            o_ = boff + j * 128
                            nc.tensor.matmul(out=pxs[:, j, :],
                                             lhsT=eTt[:, o_:o_ + 128],
                                             rhs=We[:], start=(j == 0),
                                             stop=False, skip_group_check=True)
                        if layer == 0:
                            ph = qh.tile([128, GRP], F32, tag="ph")
                            nc.tensor.matmul(out=ph[:], lhsT=wt["W1f8"][:],
                                             rhs=xet[:, boff:boff + GRP],
                                             start=True, stop=True)
                            heT = ep.tile([128, GRP], BF, tag="heT")
                            nc.scalar.activation(out=heT[:], in_=ph[:],
                                                 func=GELU, bias=wt["b1c"][:])
                            for j in range(4):
                                nc.tensor.matmul(
                                    out=pxs[:, j, :],
                                    lhsT=heT[:, j * 128:(j + 1) * 128],
                                    rhs=wt["Ws1"][:], start=False, stop=False)
                        else:
                            nc.tensor.matmul(out=pxs[:], lhsT=wt["identb"][:],
                                             rhs=xsg[:, xoff:xoff + 4, :],
                                             start=False, stop=False)
                        for j in range(4):
                            w_j = chunks[j][0]
                            if w_j < 0:
                                w_j = 0  # pad chunk: P4 row is zero anyway
                            nc.tensor.matmul(
                                out=pxs[:, j, :], lhsT=Pt[:, j, :],
                                rhs=xd_sb[:, w_j * 128:(w_j + 1) * 128],
                                start=False, stop=True,
                                skip_group_check=True)

                        mg = ep.tile([128, 4, 128], BF, tag="mg")
                        nc.scalar.activation(out=mg[:], in_=pxs[:], func=GELU)

                        for j in range(4):
                            ch = meta_[cc0 + j]
                            w = ch[0]
                            if w < 0:
                                continue
                            first, last = ch[1], ch[2]
                            if layer == 1:
                                # unit-relative overrides: a segment split
                                # across units flushes via agg_sb partials
                                if ufirst and j == 0:
                                    first = True
                                if ulast and j == jlast:
                                    last = True
                                chunk_cnt[w] += 1
                            if first:
                                active_agg[layer] = qa.tile(
                                    [128, 128], F32, tag=f"agg{layer}",
                                    name=f"agg_ps{layer}", bufs=1)
                            nc.tensor.matmul(out=active_agg[layer][:],
                                             lhsT=P4[:, j, :], rhs=mg[:, j, :],
                                             start=first, stop=last)
                            if last:
                                if layer == 0:
                                    finalize0(w, active_agg[layer])
                                else:
                                    finalize1_seg(
                                        w, chunk_cnt[w] == chunks_per_w[w],
                                        active_agg[layer])

                    # ---- L1 schedule: fixed ~4-group units (one gather op
                    # each), drained OUT OF ORDER as (epoch AG done) x (dst
                    # window finalized) eligibility allows -- avoids
                    # head-of-line blocking of later epochs behind an early
                    # epoch's high windows. Segments split across unit
                    # boundaries flush through agg_sb partials.
                    units = []          # list of dicts: g0s, maxw, ep
                    if run_l1:
                        gl = []
                        for g0 in range(0, L1, GRP):
                            cc0 = g0 // 128
                            chs = [meta1[cc0 + j] for j in range(4)]
                            ws_ = [ch[0] for ch in chs if ch[0] >= 0]
                            if not ws_:
                                continue
                            ep_i = next(gg for gg in range(NEP)
                                        if ep_start[gg] <= g0 < ep_end[gg])
                            gl.append((g0, ep_i, max(ws_)))
                        i = 0
                        while i < len(gl):
                            g0s = [gl[i][0]]
                            ep_i, maxw = gl[i][1], gl[i][2]
                            j = i + 1
                            ucap = int(os.environ.get("KERNEL_UCAP", "1"))
                            while (j < len(gl) and len(g0s) < ucap
                                   and gl[j][1] == ep_i
                                   and gl[j][0] == g0s[-1] + GRP):
                                g0s.append(gl[j][0])
                                maxw = max(maxw, gl[j][2])
                                j += 1
                            units.append(dict(g0s=g0s, maxw=maxw, ep=ep_i))
                            i = j
                    n_units = len(units)
                    done_u = [False] * n_units
                    n_drained = [0]
                    gathered = []       # FIFO of units with gathers emitted

                    def select_unit():
                        for ui in range(n_units):
                            if done_u[ui]:
                                continue
                            u = units[ui]
                            if not ag_issued[u["ep"]]:
                                continue
                            if l0_prog[0] - ag_at[u["ep"]] < ag_lead:
                                continue
                            if u["maxw"] > w_done[0]:
                                continue
                            done_u[ui] = True
                            return u
                        return None

                    gahead = int(os.environ.get("KERNEL_GAHEAD", "24"))

                    glag = int(os.environ.get("KERNEL_GLAG", "0"))

                    def drain_gathers():
                        while len(gathered) < gahead:
                            u = select_unit()
                            if u is None:
                                return
                            emit_gather(u)
                            gathered.append((u, l0_prog[0]))

                    def drain_l1(cap=2, final=False):
                        done = 0
                        while gathered and done < cap:
                            u, tick = gathered[0]
                            # wait for the unit's gather DMA to have had time
                            # to land before the PE consumes it
                            if not final and l0_prog[0] - tick < glag:
                                return
                            gathered.pop(0)
                            for i_, g0 in enumerate(u["g0s"]):
                                emit_group(1, g0, ufirst=(i_ == 0),
                                           ulast=(i_ == len(u["g0s"]) - 1))
                            n_drained[0] += 1
                            done += 1

                    for g0 in range(0, L0, GRP):
                        if g0 % (4 * GRP) == 0:
                            load_l0_block(g0)
                        emit_group(0, g0)
                        l0_prog[0] += 1
                        if run_l1:
                            drain_gathers()
                            drain_l1()
                    if run_l1:
                        w_done[0] = WCNT  # everything finalized
                        guard = 0
                        while n_drained[0] < n_units:
                            drain_gathers()
                            drain_l1(final=True)
                            l0_prog[0] += 1  # advance virtual time for ag_lead
                            guard += 1
                            assert guard < 100000, "final drain stuck"
                        assert n_drained[0] == n_units

            # ---------------- program ----------------
            phases = int(os.environ.get("KERNEL_PHASES", "3"))
            if phases >= 2:
                dense_own()
                edge_phases(run_l1=(phases >= 3))
            else:
                dense_own()
            if phases < 3:
                with tc.tile_pool(name="dbg", bufs=2) as dbp:
                    for w in range(WCNT):
                        dsb = dbp.tile([128, 128], F32, tag="dsb")
                        nc.vector.tensor_copy(
                            out=dsb[:], in_=h_own[:, w * 128:(w + 1) * 128])
                        nc.sync.dma_start(
                            out=t_out[w * 128:(w + 1) * 128, :], in_=dsb[:])

    nc.finalize()
    return nc


_CACHE = {}


def _get_program(meta):
    key = (meta["L0"], meta["L1"], tuple(meta["meta0"]), tuple(meta["meta1"]),
           tuple(meta["ep_start"]), tuple(meta["ep_end"]))
    if key not in _CACHE:
        _CACHE[key] = _build(meta)
    return _CACHE[key]


def kernel(**inputs):
    shared, per_core, meta = _prep(inputs)
    nc = _get_program(meta)
    in_maps = []
    for c in range(NC):
        m = dict(shared)
        m.update(per_core[c])
        in_maps.append(m)
    trace = os.environ.get("KERNEL_TRACE", "0") == "1"
    kw = {}
    if trace:
        kw = dict(trace=True, trace_kwargs={"title": "gnn_mp_v3"})
    res = run_bass_kernel_spmd(nc, in_maps, core_ids=list(range(NC)), **kw)
    if trace and res.exec_time_ns is not None:
        print(f"HW exec time: {res.exec_time_ns} ns")
        if res.instructions_and_trace:
            print("trace:", res.instructions_and_trace[1])
    out = np.concatenate([res.results[c]["out"] for c in range(NC)], axis=0)
    return np.ascontiguousarray(out[:N]).astype(np.float32)
